# revision 6
# baseline (speedup 1.0000x reference)
"""BranchedLinear (block-diagonal grouped GEMM) Trainium2 kernel.

Reference computation:
    x:[N, 64*32] -> reshape [N, 64, 32];  out[n,b,:] = x[n,b,:] @ W[b] + bias[b]
    -> reshape [N, 64*32]

Strategy (8 NeuronCores, data-parallel on batch):
  * Shard batch N=16384 across 8 cores (2048 rows each).
  * Host-side prep (numpy, cheap):
      - x shard is pre-transposed into feature-major tiles
        xt[c, g, p, n'] = x[512c + n', 128g + p]  (c=chunk, g=128-feature group)
        so every DMA is fully contiguous and the contraction dim (features)
        lands on SBUF partitions without any on-chip transpose.
      - W [64,32,32] is packed into a block-diagonal [128, 2048] matrix:
        each 128-col group g holds branches 4g..4g+3 as 32x32 diagonal blocks.
        A single K=128 matmul then computes 4 branches at once.
      - bias is broadcast to [128, 2048].
  * On-chip per core: for each 128-row slice of the batch, 16 fp32 matmuls
    (one per feature group) write natural-orientation [128n, 128f_out] tiles
    into PSUM; DVE does a fused bias-add + PSUM->SBUF copy; contiguous 1 MiB
    stores to HBM. Everything except the DMAs hides under the ~33 MiB/core
    memory roofline.
"""

import numpy as np

# Problem shape (hardcoded per contract)
BATCH = 16384
NUM_BRANCHES = 64
IN_FEATURES = 32
OUT_FEATURES = 32
D = NUM_BRANCHES * IN_FEATURES  # 2048

NUM_CORES = 8
SHARD = BATCH // NUM_CORES  # 2048 rows per core
P = 128
GROUPS = D // P  # 16 feature groups (4 branches each)
BRANCH_PER_GROUP = P // IN_FEATURES  # 4

# per-core tiling
CHUNKS = 4  # batch chunks per core
CHUNK_N = SHARD // CHUNKS  # 512
SUB = CHUNK_N // P  # 4 x 128-row subtiles per chunk

_NC_CACHE = {}


def _build_bass(chunks=CHUNKS, chunk_n=CHUNK_N):
    import concourse.mybir as mybir
    from concourse import bacc
    from concourse.tile import TileContext

    f32 = mybir.dt.float32
    sub = chunk_n // P
    shard = chunks * chunk_n

    nc = bacc.Bacc("TRN2", target_bir_lowering=False, debug=False)
    xt = nc.dram_tensor("xt", [chunks, GROUPS, P, chunk_n], f32, kind="ExternalInput")
    wbd = nc.dram_tensor("wbd", [P, D], f32, kind="ExternalInput")
    bias = nc.dram_tensor("bias", [P, D], f32, kind="ExternalInput")
    out = nc.dram_tensor("out", [shard, D], f32, kind="ExternalOutput")

    with TileContext(nc) as tc:
        with (
            tc.tile_pool(name="wpool", bufs=1) as wpool,
            tc.tile_pool(name="xpool", bufs=24) as xpool,
            tc.tile_pool(name="opool", bufs=4) as opool,
            tc.tile_pool(name="pspool", bufs=2, space="PSUM") as pspool,
        ):
            w_sb = wpool.tile([P, D], f32, tag="w")
            nc.sync.dma_start(out=w_sb[:], in_=wbd[:])
            b_sb = wpool.tile([P, D], f32, tag="b")
            nc.sync.dma_start(out=b_sb[:], in_=bias[:])

            for c in range(chunks):
                xs = []
                for g in range(GROUPS):
                    t = xpool.tile([P, chunk_n], f32, tag="xt")
                    nc.sync.dma_start(out=t[:], in_=xt[:][c, g])
                    xs.append(t)
                for s in range(sub):
                    o_sb = opool.tile([P, D], f32, tag="o")
                    # one 4-bank PSUM tile holds all 16 groups' outputs for
                    # this 128-row slice
                    ps = pspool.tile([P, D], f32, tag="ps")
                    for g in range(GROUPS):
                        nc.tensor.matmul(
                            ps[:, g * P : (g + 1) * P],
                            xs[g][:, s * P : (s + 1) * P],
                            w_sb[:, g * P : (g + 1) * P],
                            start=True,
                            stop=True,
                        )
                    # fused bias add + PSUM->SBUF copyback
                    nc.vector.tensor_add(
                        out=o_sb[:],
                        in0=ps[:],
                        in1=b_sb[:],
                    )
                    row0 = c * chunk_n + s * P
                    nc.sync.dma_start(out=out[:][row0 : row0 + P, :], in_=o_sb[:])
    nc.compile()
    return nc


def _get_nc(chunks=CHUNKS, chunk_n=CHUNK_N):
    key = (chunks, chunk_n)
    if key not in _NC_CACHE:
        _NC_CACHE[key] = _build_bass(chunks, chunk_n)
    return _NC_CACHE[key]


def _pack_wbd(W):
    """[64, 32, 32] -> block-diagonal [128, 2048]."""
    W = np.asarray(W, np.float32)
    wbd = np.zeros((P, D), np.float32)
    for g in range(GROUPS):
        for j in range(BRANCH_PER_GROUP):
            b = g * BRANCH_PER_GROUP + j
            r0 = j * IN_FEATURES
            c0 = g * P + j * OUT_FEATURES
            wbd[r0 : r0 + IN_FEATURES, c0 : c0 + OUT_FEATURES] = W[b]
    return wbd


def _pack_xt(shard, chunks=CHUNKS, chunk_n=CHUNK_N):
    """[shard_n, 2048] -> [chunks, GROUPS, 128, chunk_n] feature-major tiles."""
    return np.ascontiguousarray(
        shard.reshape(chunks, chunk_n, GROUPS, P).transpose(0, 2, 3, 1)
    )


def kernel(x, W, b):
    from concourse.bass_utils import run_bass_kernel_spmd

    x = np.asarray(x, np.float32)
    wbd = _pack_wbd(W)
    bias = np.ascontiguousarray(
        np.broadcast_to(np.asarray(b, np.float32).reshape(1, D), (P, D))
    )

    nc = _get_nc()
    in_maps = []
    for i in range(NUM_CORES):
        shard = x[i * SHARD : (i + 1) * SHARD]
        in_maps.append({"xt": _pack_xt(shard), "wbd": wbd, "bias": bias})

    res = run_bass_kernel_spmd(nc, in_maps, core_ids=list(range(NUM_CORES)))
    return np.concatenate([r["out"] for r in res.results], axis=0)
